# revision 39
# baseline (speedup 1.0000x reference)
"""AWPLoss kernel for Trainium2 (8 NeuronCores, pure data-parallel over batch).

Reference semantics (nn_AWPLoss): sample an alignment a ~ Categorical(log_probs)
per (b, t), clone it (f_prop = identity), and compute
    loss = mean(relu(lambda + log_probs[b,t,a] - log_probs[b,t,a_clone])).
Because the alignment is cloned, original_prob and enhanced_prob are the same
tensor, and the loss reduces to mean(relu(fl(lambda + p) - p)) where p is the
log-prob of the chosen class — the value depends on the sample only through
float32 rounding of (lambda + p) - p, i.e. at the ~1e-5 relative level.

This kernel therefore streams all of log_probs through SBUF (the memory
roofline for this problem), takes the greedy sample p = max_c log_probs[b,t,c]
per row (the mode of the categorical — any choice of sample agrees with the
reference to ~2e-5 relative), computes relu((lambda + p) - p) in float32, and
accumulates. Batch B=64 is sharded 8 ways; per-core partial sums are combined
on the host.

Per-core layout: shard [8, 4096, 128] viewed flat as [32768 rows, 128 classes].
Partition p of SBUF owns rows [p*256, (p+1)*256); each tile moves RT rows per
partition (contiguous RT*512 bytes per partition per DMA).
"""

import numpy as np

B, T, C = 64, 4096, 128
N_CORES = 8
B_PER_CORE = B // N_CORES            # 8
ROWS_PER_CORE = B_PER_CORE * T       # 32768
ROWS_PER_PART = ROWS_PER_CORE // 128  # 256 rows owned by each SBUF partition
# Rows-per-partition per tile. Front-loaded: the big first tile hides the DMA
# fill behind the first reduce, and the gentle taper keeps the stream ahead
# of the back-to-back DVE reduce chain; the tiny last tiles minimize the
# exposed final reduce.
SIZES = [136, 56, 28, 16, 10, 6, 4]
assert sum(SIZES) == ROWS_PER_PART
N_TILES = len(SIZES)
R0 = SIZES[0]
LAMBDA = 0.01

_NC_CACHE = {}


def _build_bass():
    """Raw Bass (no TileContext): avoids Tile's entry EVSEM barrier and its
    kernel-tail drain + butterfly + sem-reset (~13 us of fixed overhead).

    Two engines: SP issues the stream DMAs and the final store, DVE reduces
    each tile. One semaphore per tile: HWDGE completions signal per tile and
    the SP ring is FIFO, so tiles land strictly in order.
    """
    from contextlib import ExitStack

    import concourse.bass as bass
    import concourse.mybir as mybir

    nc = bass.Bass()
    x = nc.dram_tensor(
        "x", [ROWS_PER_CORE, C], mybir.dt.float32, kind="ExternalInput"
    )
    partial = nc.dram_tensor(
        "partial", [128, ROWS_PER_PART], mybir.dt.float32, kind="ExternalOutput"
    )

    # [128, ROWS_PER_PART*C]: partition p's line = rows p*256..(p+1)*256 flat.
    xv = x[:, :].rearrange("(p b) c -> p (b c)", p=128)

    with ExitStack() as ctx:
        # Manual BassBlock so the exit can skip the ~5 us all-engine EVSEM
        # barrier: with only SP and DVE active and the store already waited
        # on, NEFF completion (all queues drained) needs no extra barrier.
        block = bass.BassBlock(nc, "b0")
        block.__enter__()
        dve_sem = ctx.enter_context(nc.semaphore("dve_sem"))
        out_sem = ctx.enter_context(nc.semaphore("out_sem"))
        tile_sems = [
            ctx.enter_context(nc.semaphore(f"ts{t}")) for t in range(N_TILES)
        ]
        tiles = [
            ctx.enter_context(
                nc.sbuf_tensor(f"s{t}", [128, SIZES[t] * C], mybir.dt.bfloat16)
            )
            for t in range(N_TILES)
        ]
        # Class-fold scratches (shared across tiles, sized for the largest).
        scrA = ctx.enter_context(
            nc.sbuf_tensor("scrA", [128, R0 * 64], mybir.dt.bfloat16)
        )
        scrB = ctx.enter_context(
            nc.sbuf_tensor("scrB", [128, R0 * 32], mybir.dt.bfloat16)
        )
        # One pmax column range per tile: reduces write disjoint slices, so
        # consecutive reduces need no same-engine sem wait between them.
        pmax = ctx.enter_context(
            nc.sbuf_tensor("pmax", [128, ROWS_PER_PART], mybir.dt.float32)
        )
        dbuf = ctx.enter_context(
            nc.sbuf_tensor("dbuf", [128, ROWS_PER_PART], mybir.dt.float32)
        )

        offs = [sum(SIZES[:t]) for t in range(N_TILES)]
        # DVE progress ticks on dve_sem (every DVE op increments it; dependent
        # same-engine ops must wait — the DVE pipeline needs explicit sem sync
        # for RAW/WAR, same as Tile emits):
        #   tile t: reduce -> 3t+1, stt -> 3t+2, acc -> 3t+3.
        # No issue throttle: every tile has its own buffer and the SWDGE
        # queue is FIFO, so tiles land strictly in order. SWDGE (gpsimd)
        # performs the f32->bf16 cast inline during the DMA.
        @block.gpsimd
        def _(g: bass.BassEngine):
            for t in range(N_TILES):
                g.dma_start(
                    out=tiles[t][:, :],
                    in_=xv[:, offs[t] * C : (offs[t] + SIZES[t]) * C],
                ).then_inc(tile_sems[t], 16)

        @block.sync
        def _(sync: bass.BassEngine):
            sync.wait_ge(dve_sem, 3 * N_TILES + 1)
            sync.dma_start(out=partial[:, :], in_=dbuf[:, :]).then_inc(out_sem, 16)
            sync.wait_ge(out_sem, 16)

        @block.vector
        def _(vector: bass.BassEngine):
            # Per tile: fold classes 128->64->32 with bf16 tensor_tensor max
            # (2x DVE mode), then a 1x reduce over the remaining 32 — ~40%
            # less DVE time than reducing 128 directly. Ticks: tile t ->
            # 3t+1 (fold1), 3t+2 (fold2), 3t+3 (reduce).
            for t in range(N_TILES):
                rt = SIZES[t]
                t3 = tiles[t][:, :].rearrange("p (r c) -> p r c", c=C)
                a3 = scrA[:, : rt * 64].rearrange("p (r c) -> p r c", c=64)
                b3 = scrB[:, : rt * 32].rearrange("p (r c) -> p r c", c=32)
                vector.wait_ge(tile_sems[t], 16)
                if t > 0:
                    # WAR on scrA vs previous tile's fold2 read.
                    vector.wait_ge(dve_sem, 3 * t - 1)
                nc.vector.tensor_tensor(
                    out=a3,
                    in0=t3[:, :, 0:64],
                    in1=t3[:, :, 64:128],
                    op=mybir.AluOpType.max,
                ).then_inc(dve_sem, 1)
                vector.wait_ge(dve_sem, 3 * t + 1)
                nc.vector.tensor_tensor(
                    out=b3,
                    in0=a3[:, :, 0:32],
                    in1=a3[:, :, 32:64],
                    op=mybir.AluOpType.max,
                ).then_inc(dve_sem, 1)
                vector.wait_ge(dve_sem, 3 * t + 2)
                nc.vector.reduce_max(
                    out=pmax[:, offs[t] : offs[t] + SIZES[t]],
                    in_=b3,
                    axis=mybir.AxisListType.X,
                ).then_inc(dve_sem, 1)
            # One vectorized epilogue over all 256 rows/partition:
            # d = (pmax + LAMBDA) - pmax in float32 (the reference's
            # evaluation order); relu runs on the host.
            vector.wait_ge(dve_sem, 3 * N_TILES)
            nc.vector.scalar_tensor_tensor(
                out=dbuf[:, :],
                in0=pmax[:, :],
                scalar=LAMBDA,
                in1=pmax[:, :],
                op0=mybir.AluOpType.add,
                op1=mybir.AluOpType.subtract,
            ).then_inc(dve_sem, 1)
            # relu(d) runs on the host during unsharding (numerically it is
            # a no-op here: fl(LAMBDA + p) >= p for all finite p with
            # |p| << LAMBDA * 2^24, which log-softmax outputs satisfy).

        # Barrier-free block finalize (BassBlock.__exit__ minus the
        # all_engine_barrier).
        for engine, last_body in block.last_body.items():
            with nc.body(
                last_body, parent=nc.cur_bb, allow_existing_parent=True
            ):
                engine.br(block.end_bb)
        nc.switch_bb(block.end_bb)

    _use_add_imm_sem_updates(nc)
    _strip_init_barrier(nc)
    return nc


def _strip_init_barrier(nc):
    """Drop Bass-init const-AP memsets and the init all-engine barrier from
    the 'main' block. Nothing in this kernel reads the const APs, and the
    engines need no common start line — SP can issue the first stream DMA as
    soon as its register preamble is done."""
    for f in nc.m.functions:
        for blk in f.blocks:
            if blk.name != "main":
                continue
            blk.instructions = [
                i
                for i in blk.instructions
                if type(i).__name__
                not in ("InstMemset", "InstDrain", "InstEventSemaphore")
            ]


def _use_add_imm_sem_updates(nc):
    """then_inc emits update_mode='sem-inc' (event-accelerator path); Tile
    emits 'sem-add-imm', which measures ~0.9 us faster per DVE op on HW.
    Rewrite in place."""
    import concourse.mybir as mybir

    ok = ("InstTensorReduce", "InstTensorScalarPtr", "InstMemSet", "InstDMACopy")
    for f in nc.m.functions:
        for blk in f.blocks:
            for inst in blk.instructions:
                if type(inst).__name__ not in ok:
                    continue
                si = inst.sync_info
                if si and si.on_update:
                    si.on_update = [
                        mybir.SyncUpdate(
                            sync_type=u.sync_type,
                            id=u.id,
                            ant_name=u.ant_name,
                            update_mode="sem-add-imm",
                            update_value=u.update_value,
                            update_reg=u.update_reg,
                        )
                        if u.update_mode == "sem-inc"
                        else u
                        for u in si.on_update
                    ]
                    inst.sync_info = si


def _get_nc():
    if "nc" not in _NC_CACHE:
        _NC_CACHE["nc"] = _build_bass()
    return _NC_CACHE["nc"]


def _run(lp, trace=False):
    from concourse.bass_utils import run_bass_kernel_spmd

    in_maps = [
        {"x": np.ascontiguousarray(lp[c * B_PER_CORE : (c + 1) * B_PER_CORE]).reshape(
            ROWS_PER_CORE, C
        )}
        for c in range(N_CORES)
    ]
    return run_bass_kernel_spmd(
        _get_nc(), in_maps, core_ids=list(range(N_CORES)), trace=trace
    )


def kernel(log_probs, targets=None, input_lengths=None, target_lengths=None):
    lp = np.asarray(log_probs, dtype=np.float32)
    assert lp.shape == (B, T, C), lp.shape
    res = _run(lp)
    total = sum(
        np.maximum(r["partial"], np.float32(0.0)).sum(dtype=np.float64)
        for r in res.results
    )
    return np.asarray(total / (B * T), dtype=np.float32)


# revision 40
# speedup vs baseline: 1.4088x; 1.4088x over previous
"""AWPLoss kernel for Trainium2 (8 NeuronCores, pure data-parallel over batch).

Reference semantics (nn_AWPLoss): sample an alignment a ~ Categorical(log_probs)
per (b, t), clone it (f_prop = identity), and compute
    loss = mean(relu(lambda + log_probs[b,t,a] - log_probs[b,t,a_clone])).
Because the alignment is cloned, original_prob and enhanced_prob are the same
tensor, and the loss reduces to mean(relu(fl(lambda + p) - p)) where p is the
log-prob of the chosen class — the value depends on the sample only through
float32 rounding of (lambda + p) - p, i.e. at the ~1e-5 relative level.

This kernel therefore streams all of log_probs through SBUF (the memory
roofline for this problem), takes the greedy sample p = max_c log_probs[b,t,c]
per row (the mode of the categorical — any choice of sample agrees with the
reference to ~2e-5 relative), computes relu((lambda + p) - p) in float32, and
accumulates. Batch B=64 is sharded 8 ways; per-core partial sums are combined
on the host.

Per-core layout: shard [8, 4096, 128] viewed flat as [32768 rows, 128 classes].
Partition p of SBUF owns rows [p*256, (p+1)*256); each tile moves RT rows per
partition (contiguous RT*512 bytes per partition per DMA).
"""

import numpy as np

B, T, C = 64, 4096, 128
N_CORES = 8
B_PER_CORE = B // N_CORES            # 8
ROWS_PER_CORE = B_PER_CORE * T       # 32768
ROWS_PER_PART = ROWS_PER_CORE // 128  # 256 rows owned by each SBUF partition
# Rows-per-partition per tile. Front-loaded: the big first tile hides the DMA
# fill behind the first reduce, and the gentle taper keeps the stream ahead
# of the back-to-back DVE reduce chain; the tiny last tiles minimize the
# exposed final reduce.
SIZES = [48, 40, 36, 32, 28, 24, 20, 16, 8, 4]
assert sum(SIZES) == ROWS_PER_PART
N_TILES = len(SIZES)
LAMBDA = 0.01

_NC_CACHE = {}


def _build_bass():
    """Raw Bass (no TileContext): avoids Tile's entry EVSEM barrier and its
    kernel-tail drain + butterfly + sem-reset (~13 us of fixed overhead).

    Two engines: SP issues the stream DMAs and the final store, DVE reduces
    each tile. One semaphore per tile: HWDGE completions signal per tile and
    the SP ring is FIFO, so tiles land strictly in order.
    """
    from contextlib import ExitStack

    import concourse.bass as bass
    import concourse.mybir as mybir

    nc = bass.Bass()
    x = nc.dram_tensor(
        "x", [ROWS_PER_CORE, C], mybir.dt.float32, kind="ExternalInput"
    )
    partial = nc.dram_tensor(
        "partial", [128, ROWS_PER_PART], mybir.dt.float32, kind="ExternalOutput"
    )

    # [128, ROWS_PER_PART*C]: partition p's line = rows p*256..(p+1)*256 flat.
    xv = x[:, :].rearrange("(p b) c -> p (b c)", p=128)

    with ExitStack() as ctx:
        # Manual BassBlock so the exit can skip the ~5 us all-engine EVSEM
        # barrier: with only SP and DVE active and the store already waited
        # on, NEFF completion (all queues drained) needs no extra barrier.
        block = bass.BassBlock(nc, "b0")
        block.__enter__()
        dve_sem = ctx.enter_context(nc.semaphore("dve_sem"))
        out_sem = ctx.enter_context(nc.semaphore("out_sem"))
        tile_sems = [
            ctx.enter_context(nc.semaphore(f"ts{t}")) for t in range(N_TILES)
        ]
        tiles = [
            ctx.enter_context(
                nc.sbuf_tensor(f"s{t}", [128, SIZES[t] * C], mybir.dt.float32)
            )
            for t in range(N_TILES)
        ]
        # One pmax column range per tile: reduces write disjoint slices, so
        # consecutive reduces need no same-engine sem wait between them.
        pmax = ctx.enter_context(
            nc.sbuf_tensor("pmax", [128, ROWS_PER_PART], mybir.dt.float32)
        )
        dbuf = ctx.enter_context(
            nc.sbuf_tensor("dbuf", [128, ROWS_PER_PART], mybir.dt.float32)
        )

        offs = [sum(SIZES[:t]) for t in range(N_TILES)]
        # DVE progress ticks on dve_sem (every DVE op increments it; dependent
        # same-engine ops must wait — the DVE pipeline needs explicit sem sync
        # for RAW/WAR, same as Tile emits):
        #   tile t: reduce -> 3t+1, stt -> 3t+2, acc -> 3t+3.
        # No issue throttle: every tile has its own buffer and the SP HWDGE
        # ring is FIFO, so tiles land strictly in order and back-to-back
        # issue keeps the ring fed — the stream is one continuous burst.
        @block.sync
        def _(sync: bass.BassEngine):
            for t in range(N_TILES):
                sync.dma_start(
                    out=tiles[t][:, :],
                    in_=xv[:, offs[t] * C : (offs[t] + SIZES[t]) * C],
                ).then_inc(tile_sems[t], 16)
            sync.wait_ge(dve_sem, N_TILES + 1)
            sync.dma_start(out=partial[:, :], in_=dbuf[:, :]).then_inc(out_sem, 16)
            sync.wait_ge(out_sem, 16)

        @block.vector
        def _(vector: bass.BassEngine):
            # Back-to-back reduces: tile t's per-row max lands in its own
            # pmax column slice (dve tick t+1).
            for t in range(N_TILES):
                vector.wait_ge(tile_sems[t], 16)
                nc.vector.reduce_max(
                    out=pmax[:, offs[t] : offs[t] + SIZES[t]],
                    in_=tiles[t][:, :].rearrange("p (r c) -> p r c", c=C),
                    axis=mybir.AxisListType.X,
                ).then_inc(dve_sem, 1)
            # One vectorized epilogue over all 256 rows/partition:
            # d = (pmax + LAMBDA) - pmax in float32 (the reference's
            # evaluation order), then relu in place.
            vector.wait_ge(dve_sem, N_TILES)
            nc.vector.scalar_tensor_tensor(
                out=dbuf[:, :],
                in0=pmax[:, :],
                scalar=LAMBDA,
                in1=pmax[:, :],
                op0=mybir.AluOpType.add,
                op1=mybir.AluOpType.subtract,
            ).then_inc(dve_sem, 1)
            # relu(d) runs on the host during unsharding (numerically it is
            # a no-op here: fl(LAMBDA + p) >= p for all finite p with
            # |p| << LAMBDA * 2^24, which log-softmax outputs satisfy).

        # Barrier-free block finalize (BassBlock.__exit__ minus the
        # all_engine_barrier).
        for engine, last_body in block.last_body.items():
            with nc.body(
                last_body, parent=nc.cur_bb, allow_existing_parent=True
            ):
                engine.br(block.end_bb)
        nc.switch_bb(block.end_bb)

    _use_add_imm_sem_updates(nc)
    _strip_init_barrier(nc)
    return nc


def _strip_init_barrier(nc):
    """Drop Bass-init const-AP memsets and the init all-engine barrier from
    the 'main' block. Nothing in this kernel reads the const APs, and the
    engines need no common start line — SP can issue the first stream DMA as
    soon as its register preamble is done."""
    for f in nc.m.functions:
        for blk in f.blocks:
            if blk.name != "main":
                continue
            blk.instructions = [
                i
                for i in blk.instructions
                if type(i).__name__
                not in ("InstMemset", "InstDrain", "InstEventSemaphore")
            ]


def _use_add_imm_sem_updates(nc):
    """then_inc emits update_mode='sem-inc' (event-accelerator path); Tile
    emits 'sem-add-imm', which measures ~0.9 us faster per DVE op on HW.
    Rewrite in place."""
    import concourse.mybir as mybir

    ok = ("InstTensorReduce", "InstTensorScalarPtr", "InstMemSet", "InstDMACopy")
    for f in nc.m.functions:
        for blk in f.blocks:
            for inst in blk.instructions:
                if type(inst).__name__ not in ok:
                    continue
                si = inst.sync_info
                if si and si.on_update:
                    si.on_update = [
                        mybir.SyncUpdate(
                            sync_type=u.sync_type,
                            id=u.id,
                            ant_name=u.ant_name,
                            update_mode="sem-add-imm",
                            update_value=u.update_value,
                            update_reg=u.update_reg,
                        )
                        if u.update_mode == "sem-inc"
                        else u
                        for u in si.on_update
                    ]
                    inst.sync_info = si


def _get_nc():
    if "nc" not in _NC_CACHE:
        _NC_CACHE["nc"] = _build_bass()
    return _NC_CACHE["nc"]


def _run(lp, trace=False):
    from concourse.bass_utils import run_bass_kernel_spmd

    in_maps = [
        {"x": np.ascontiguousarray(lp[c * B_PER_CORE : (c + 1) * B_PER_CORE]).reshape(
            ROWS_PER_CORE, C
        )}
        for c in range(N_CORES)
    ]
    return run_bass_kernel_spmd(
        _get_nc(), in_maps, core_ids=list(range(N_CORES)), trace=trace
    )


def kernel(log_probs, targets=None, input_lengths=None, target_lengths=None):
    lp = np.asarray(log_probs, dtype=np.float32)
    assert lp.shape == (B, T, C), lp.shape
    res = _run(lp)
    total = sum(
        np.maximum(r["partial"], np.float32(0.0)).sum(dtype=np.float64)
        for r in res.results
    )
    return np.asarray(total / (B * T), dtype=np.float32)
